# revision 1
# baseline (speedup 1.0000x reference)
"""Trainium2 Bass kernel for multi-scale multi-camera deformable aggregation
(Sparse4D DFA): out[b,a,g,d] = sum_{p,cam,lvl} attw * bilinear_sample(value).

Strategy (8 NeuronCores, SPMD, no collectives):
  - Shard over (batch, anchor-block): core = b*4 + q handles anchors
    [q*225, (q+1)*225) of batch b, padded to 232 = 29 groups x 8 anchors.
  - Host precomputes, per core: an fp16 "interleaved pair" value table
    (row (cam,h,w) = [v[h,w,ch], v[h,w+1,ch]] interleaved per channel, so one
    gathered row covers a (w,w+1) pair for all 256 channels), int16 gather
    indices in the SWDGE wrapped layout, and fp16 per-row scale tables
    scale[row,(g8,pos)] = attn_w[sample,g8] * wh(slot) * ww(pos).
  - Device, per (group of 8 anchors, campair): dma_gather 1664 rows
    (8 anchors x 2 cams x 4 lvls x 13 pts x 2 h-slots) of 512 fp16;
    DVE multiplies by broadcast scales; 13 matmuls against a constant 0/1
    selection matrix accumulate rows into psum[8 anchors, 512].
  - psum -> SBUF -> DRAM [232, 512]; host folds the (w0,w1) lane pairs and
    assembles the full [2, 900, 256] f32 output.
"""
import os
import functools
import numpy as np

import concourse.bacc as bacc
import concourse.mybir as mybir
from concourse.tile import TileContext
from concourse.bass_utils import run_bass_kernel_spmd

# nuScenes-style config (hardcoded per problem spec)
SPATIAL = [(64, 176), (32, 88), (16, 44), (8, 22)]
STARTS = [0, 11264, 14080, 14784]
PER_CAM = 14960
NCAMS, LVLS, PTS, GROUPS, EMBED = 6, 4, 13, 8, 256
BS, ANCHORS = 2, 900
NCORES = 8
APC = 225          # anchors per core
NG = 29            # anchor groups of 8 per core
APAD = NG * 8      # 232, padded anchors per core
CP = 3             # camera pairs
ROWS_PER_A = 2 * LVLS * PTS * 2   # rows per anchor per campair = 208
NROW = 8 * ROWS_PER_A             # rows per gather call = 1664
KT = NROW // 128                  # sbuf tiles per call = 13
TROWS = 2 * PER_CAM               # value-table rows per campair = 29920

F16 = mybir.dt.float16
F32 = mybir.dt.float32
I16 = mybir.dt.int16


@functools.lru_cache(maxsize=2)
def _build_program(reps: int):
    nc = bacc.Bacc("TRN2", target_bir_lowering=False, debug=False,
                   num_devices=1, enable_asserts=False)
    vt = nc.dram_tensor("vt", [CP * TROWS, 512], F16, kind="ExternalInput").ap()
    idx = nc.dram_tensor("idx", [NG, CP, 128, NROW // 16], I16,
                         kind="ExternalInput").ap()
    sw = nc.dram_tensor("sw", [NG, CP, 128, KT * 16], F16,
                        kind="ExternalInput").ap()
    sel = nc.dram_tensor("sel", [128, KT * 8], F16, kind="ExternalInput").ap()
    out = nc.dram_tensor("out", [APAD, 512], F32, kind="ExternalOutput").ap()

    with TileContext(nc) as tc:
        with (
            tc.tile_pool(name="const", bufs=1) as cpool,
            tc.tile_pool(name="idxp", bufs=4) as idxp,
            tc.tile_pool(name="swp", bufs=4) as swp,
            tc.tile_pool(name="gp", bufs=3) as gp,
            tc.tile_pool(name="tp", bufs=3) as tp,
            tc.tile_pool(name="psp", bufs=4, space="PSUM") as psp,
            tc.tile_pool(name="op", bufs=4) as op,
        ):
            sel_t = cpool.tile([128, KT * 8], F16)
            nc.sync.dma_start(out=sel_t[:], in_=sel[:])

            for rep in range(reps):
                for g in range(NG):
                    ps = psp.tile([8, 512], F32, space="PSUM")
                    for c in range(CP):
                        idx_t = idxp.tile([128, NROW // 16], I16)
                        nc.sync.dma_start(out=idx_t[:], in_=idx[g, c])
                        s_t = swp.tile([128, KT * 16], F16)
                        nc.sync.dma_start(out=s_t[:], in_=sw[g, c])
                        g_t = gp.tile([128, KT * 512], F16)
                        nc.gpsimd.dma_gather(
                            g_t[:].rearrange("p (k e) -> p k e", e=512),
                            vt[c * TROWS:(c + 1) * TROWS, :],
                            idx_t[:],
                            NROW, NROW, 512,
                            single_packet=False,
                        )
                        t_t = tp.tile([128, KT * 512], F16)
                        for k in range(KT):
                            nc.vector.tensor_tensor(
                                out=t_t[:, k * 512:(k + 1) * 512].rearrange(
                                    "p (g d s) -> p g d s", g=8, d=32, s=2),
                                in0=g_t[:, k * 512:(k + 1) * 512].rearrange(
                                    "p (g d s) -> p g d s", g=8, d=32, s=2),
                                in1=s_t[:, k * 16:(k + 1) * 16].rearrange(
                                    "p (g s) -> p g s", g=8, s=2
                                ).unsqueeze(2).to_broadcast([128, 8, 32, 2]),
                                op=mybir.AluOpType.mult,
                            )
                        for k in range(KT):
                            nc.tensor.matmul(
                                ps[:],
                                sel_t[:, k * 8:(k + 1) * 8],
                                t_t[:, k * 512:(k + 1) * 512],
                                start=(c == 0 and k == 0),
                                stop=(c == CP - 1 and k == KT - 1),
                            )
                    o_t = op.tile([8, 512], F32)
                    nc.scalar.copy(out=o_t[:], in_=ps[:])
                    nc.sync.dma_start(out=out[g * 8:(g + 1) * 8, :], in_=o_t[:])
    nc.compile()
    return nc


def _prep_value_tables(value: np.ndarray):
    """value [2, 89760, 256] f32 -> per-batch fp16 interleaved tables
    [89760 rows, 512] where row (cam,h,w) = interleave(v[h,w,:], v[h,w+1,:])."""
    v = np.ascontiguousarray(value).reshape(BS, NCAMS, PER_CAM, EMBED)
    tables = []
    for b in range(BS):
        vb = v[b].astype(np.float16)
        pair = np.zeros((NCAMS, PER_CAM, EMBED, 2), np.float16)
        pair[..., 0] = vb
        for lvl in range(LVLS):
            H, W = SPATIAL[lvl]
            s = STARTS[lvl]
            blk = vb[:, s:s + H * W].reshape(NCAMS, H, W, EMBED)
            sh = pair[:, s:s + H * W, :, 1].reshape(NCAMS, H, W, EMBED)
            sh[:, :, :W - 1] = blk[:, :, 1:]
        tables.append(pair.reshape(NCAMS * PER_CAM, 512))
    return tables


def _prep_core(loc: np.ndarray, attw: np.ndarray):
    """loc [APC,13,6,2], attw [APC,13,6,4,8] (one core's slice, f32) ->
    (idx [NG,CP,128,104] i16, sw [NG,CP,128,208] f16)."""
    locp = np.zeros((APAD, PTS, NCAMS, 2), np.float32)
    locp[:APC] = loc
    attp = np.zeros((APAD, PTS, NCAMS, LVLS, GROUPS), np.float32)
    attp[:APC] = attw

    Hs = np.array([h for h, w in SPATIAL], np.float32)
    Ws = np.array([w for h, w in SPATIAL], np.float32)
    Wi = Ws.astype(np.int32)
    st = np.array(STARTS, np.int32)

    w = locp[..., 0:1] * Ws - 0.5      # [A,P,C,L]
    h = locp[..., 1:2] * Hs - 0.5
    hs = np.clip(np.floor(h), 0, Hs - 2).astype(np.int32)
    ws = np.clip(np.floor(w), 0, Ws - 2).astype(np.int32)
    wh = np.stack([np.clip(1.0 - np.abs(h - hs), 0, 1),
                   np.clip(1.0 - np.abs(h - (hs + 1)), 0, 1)], -1)   # [A,P,C,L,2]
    ww = np.stack([np.clip(1.0 - np.abs(w - ws), 0, 1),
                   np.clip(1.0 - np.abs(w - (ws + 1)), 0, 1)], -1)
    cam_off = (np.arange(NCAMS, dtype=np.int32) % 2)[None, None, :, None] * PER_CAM
    idx0 = cam_off + st[None, None, None, :] + hs * Wi[None, None, None, :] + ws
    idxs = np.stack([idx0, idx0 + Wi[None, None, None, :]], -1)      # [A,P,C,L,2]

    # scale[A,P,C,L,s,g8,pos] = attw[...,g8] * wh[...,s] * ww[...,pos]
    scale = (attp[:, :, :, :, None, :, None]
             * wh[..., :, None, None]
             * ww[..., None, None, :]).astype(np.float16)

    def reorder(x, tail):
        # [A,P,C,L,*tail] -> [NG, CP, (al cl lvl pt s...), *tail']
        x = x.reshape(NG, 8, PTS, CP, 2, LVLS, *tail)
        x = x.transpose(0, 3, 1, 4, 5, 2, *range(6, 6 + len(tail)))
        return x

    idx_r = reorder(idxs, (2,)).reshape(NG, CP, NROW)
    sw_r = reorder(scale, (2, 8, 2)).reshape(NG, CP, NROW, 16)

    # wrapped idx layout: i -> [i%16 (+16*rep), i//16]
    idx_w = idx_r.reshape(NG, CP, NROW // 16, 16).transpose(0, 1, 3, 2)
    idx_t = np.tile(idx_w, (1, 1, 8, 1)).astype(np.int16)            # [NG,CP,128,104]
    # scale tile layout: i -> [i%128, i//128, :]
    sw_t = sw_r.reshape(NG, CP, KT, 128, 16).transpose(0, 1, 3, 2, 4)
    return idx_t, np.ascontiguousarray(sw_t).reshape(NG, CP, 128, KT * 16)


def _sel_matrix():
    sel = np.zeros((128, KT, 8), np.float16)
    for k in range(KT):
        for p in range(128):
            sel[p, k, (k * 128 + p) // ROWS_PER_A] = 1.0
    return sel.reshape(128, KT * 8)


def kernel(value, input_spatial_shapes, input_level_start_index,
           sampling_locations, attention_weights):
    value = np.asarray(value, dtype=np.float32)
    loc = np.asarray(sampling_locations, dtype=np.float32)
    attw = np.asarray(attention_weights, dtype=np.float32)

    tables = _prep_value_tables(value)
    sel = _sel_matrix()

    in_maps = []
    for core in range(NCORES):
        b, q = divmod(core, 4)
        sl = slice(q * APC, (q + 1) * APC)
        idx_t, sw_t = _prep_core(loc[b, sl], attw[b, sl])
        in_maps.append({"vt": tables[b], "idx": idx_t, "sw": sw_t, "sel": sel})

    reps = int(os.environ.get("DFA_REPS", "1"))
    nc = _build_program(reps)
    res = run_bass_kernel_spmd(nc, in_maps, core_ids=list(range(NCORES)))

    out = np.zeros((BS, ANCHORS, EMBED), np.float32)
    for core in range(NCORES):
        b, q = divmod(core, 4)
        r = res.results[core]["out"][:APC]                  # [225, 512]
        out[b, q * APC:(q + 1) * APC] = r.reshape(APC, EMBED, 2).sum(-1)
    return out
